# revision 6
# baseline (speedup 1.0000x reference)
"""Trainium2 Bass kernel: grouped-pointwise FFN with channel shuffle.

Computes (per batch b, all ops pointwise in T):
    h   = W1_grouped @ (x * mask) + b1          # G=4 block-diagonal GEMM
    h   = channel_shuffle(h, G)
    h   = gelu(h)                               # exact erf gelu
    out = (W2_grouped @ h + b2) * mask

Sharding: data-parallel over batch B=16 across 8 cores (2 batches/core).
Weights are replicated; no collectives.

The spec pins x_mask to all-ones, so the kernel drops the mask path on
device; if a caller ever passes a non-trivial mask it is applied exactly
on the host (x*mask pre, out*mask post) which commutes with the kernel.

Engine budget per core (pace analysis):
  ACT (gelu, 1.2GHz, dtype-independent): 64 ops x [128,1024] ~ 68.5us  <- pace
  PE  (256 matmuls x 512 free, bf16 1cyc/row @2.4GHz): ~54.6us
  DVE (GEMM2 drain +bias): 32 x [128,512] ~ 22.5us
  DMA (bf16 in 5.2MB, f32 out 8.4MB): ~41us wire
ACT is the critical engine; the schedule keeps it streaming back-to-back:
GEMM1 halves feed a 3-buf [128,1024] PSUM pool (6 banks), GEMM2 uses a
2-buf [128,512] pool (2 banks), GEMM2 of iteration i interleaves with
GEMM1 of iteration i+1 (lag 1) except the last iteration which
interleaves within itself to shorten the tail.

Channel shuffle is free: GEMM2's weight blocks are pre-gathered on the
host so GEMM2 group g2 contracts directly over GEMM1's (g, m=g2) tiles.

All matmul operands are bf16 (same PE rate as fp32r; half the DMA bytes
and SBUF); PSUM stays fp32, gelu output h is bf16, final out fp32.
"""

import numpy as np

import concourse.mybir as mybir
import concourse.tile as tile
from concourse import bacc
from concourse import bass_utils

F32 = mybir.dt.float32
BF16 = mybir.dt.bfloat16

N_CORES = 8
B, CIN, T = 16, 512, 2048
H, COUT, G = 2048, 512, 4
BPC = B // N_CORES        # batches per core
MB = (H // G) // 128      # 4 output-channel blocks per group in GEMM1
CH = 512                  # matmul free dim (1 PSUM bank)
AW = 1024                 # ACT op width (2 PSUM banks)

MM_DT = BF16

_compiled = {}


def _build(mm_dt):
    nc = bacc.Bacc(
        "TRN2", target_bir_lowering=False, debug=False, num_devices=N_CORES
    )
    xs = nc.dram_tensor("xs", [BPC * G, 128, T], mm_dt, kind="ExternalInput").ap()
    # wpk cols: w1t [(m, g, o)-major, 2048] then w2t [(g2, g, o)-major, 2048]
    wpk = nc.dram_tensor("wpk", [128, 2 * G * MB * 128], mm_dt, kind="ExternalInput").ap()
    # bpk cols: b1t [m*G+g, 16] then b2t [g2, 4]
    bpk = nc.dram_tensor("bpk", [128, G * MB + G], F32, kind="ExternalInput").ap()
    outs = nc.dram_tensor("outs", [BPC * G, 128, T], F32, kind="ExternalOutput").ap()

    with tile.TileContext(nc) as tc:
        with (
            tc.tile_pool(name="consts", bufs=1) as cpool,
            tc.tile_pool(name="xp", bufs=BPC * G) as xpool,
            tc.tile_pool(name="hp", bufs=2 * G) as hpool,
            tc.tile_pool(name="op", bufs=2) as opool,
            tc.tile_pool(name="ps1p", bufs=3, space="PSUM") as ps1pool,
            tc.tile_pool(name="ps2p", bufs=2, space="PSUM") as ps2pool,
        ):
            w_sb = cpool.tile([128, 2 * G * MB * 128], mm_dt)
            b_sb = cpool.tile([128, G * MB + G], F32)
            wup = cpool.tile([1, CH], mm_dt)
            x_sb = [[None] * G for _ in range(BPC)]

            # head DMAs: everything first-need goes on the sync HWDGE ring
            # in small chunks so GEMM1 matmuls unblock progressively;
            # x(0,0) chunks lead, weights for m=0 next.
            nc.sync.dma_start(w_sb[:, 0 : G * 128], wpk[:, 0 : G * 128])
            xt00 = xpool.tile([128, T], mm_dt, tag="x", name="xt")
            for c in range(T // CH):
                nc.sync.dma_start(
                    xt00[:, c * CH : (c + 1) * CH], xs[0][:, c * CH : (c + 1) * CH]
                )
            x_sb[0][0] = xt00
            nc.sync.dma_start(b_sb, bpk)
            nc.sync.dma_start(
                w_sb[:, G * 128 : G * MB * 128], wpk[:, G * 128 : G * MB * 128]
            )
            for g in range(1, G):
                xt = xpool.tile([128, T], mm_dt, tag="x", name="xt")
                nc.sync.dma_start(xt, xs[g])
                x_sb[0][g] = xt
            nc.sync.dma_start(
                w_sb[:, G * MB * 128 :], wpk[:, G * MB * 128 :]
            )

            # PE p-state warm-up while the head DMAs stream: memset-fed
            # K=1 matmuls, no data dependencies.
            nc.gpsimd.memset(wup.bitcast(mybir.dt.uint16), 0x3F80)
            wps = ps2pool.tile([128, CH], F32, tag="ps2", name="wps")
            for _ in range(8):
                nc.tensor.matmul(wps, wup[:, 0:128], wup, start=True, stop=True)

            def load_x(b, g):
                xt = xpool.tile([128, T], mm_dt, tag="x", name="xt")
                nc.sync.dma_start(xt, xs[b * G + g])
                x_sb[b][g] = xt

            def g1_half(b, m, g, half, ht):
                # one [128,1024] PSUM tile: 2 matmuls + fused gelu/bias
                ps1 = ps1pool.tile([128, AW], F32, tag="ps1", name="ps1")
                w_ap = w_sb[:, (m * G + g) * 128 : (m * G + g + 1) * 128]
                for c2 in range(AW // CH):
                    c = half * (AW // CH) + c2
                    nc.tensor.matmul(
                        ps1[:, c2 * CH : (c2 + 1) * CH],
                        w_ap,
                        x_sb[b][g][:, c * CH : (c + 1) * CH],
                        start=True, stop=True,
                    )
                nc.scalar.activation(
                    ht[:, half * AW : (half + 1) * AW],
                    ps1,
                    mybir.ActivationFunctionType.Gelu,
                    bias=b_sb[:, m * G + g : m * G + g + 1],
                    scale=1.0,
                )

            def g2_chunk(b, g2, hts, ot, c, fin=False):
                cs = slice(c * CH, (c + 1) * CH)
                ps2 = ps2pool.tile([128, CH], F32, tag="ps2", name="ps2")
                for g in range(G):
                    wo = G * MB * 128 + (g2 * G + g) * 128
                    nc.tensor.matmul(
                        ps2,
                        w_sb[:, wo : wo + 128],
                        hts[g][:, cs],
                        start=(g == 0), stop=(g == G - 1),
                    )
                # out = psum + b2 (per-partition scalar) on DVE
                nc.vector.tensor_scalar_add(
                    ot[:, cs],
                    ps2,
                    b_sb[:, G * MB + g2 : G * MB + g2 + 1],
                )
                if fin:
                    # tail: per-chunk stores on the (idle, HWDGE) sync ring
                    nc.sync.dma_start(outs[b * G + g2][:, cs], ot[:, cs])
                elif c % 2 == 1:  # steady state: store half-tiles
                    os_ = slice((c - 1) * CH, (c + 1) * CH)
                    nc.gpsimd.dma_start(outs[b * G + g2][:, os_], ot[:, os_])

            # pipeline: GEMM2 of iter i-1 interleaves with GEMM1 of iter i;
            # the last iteration interleaves its own GEMM2 (after the h
            # halves it needs) to shorten the tail.
            NIT = BPC * MB
            prev = None
            for it in range(NIT):
                b, m = divmod(it, MB)
                last = it == NIT - 1
                hts = [
                    hpool.tile([128, T], mm_dt, tag="h", name="ht")
                    for _ in range(G)
                ]
                if prev is not None:
                    pot = opool.tile([128, T], F32, tag="o", name="pot")
                if not last:
                    for g in range(G):
                        g1_half(b, m, g, 0, hts[g])
                        g1_half(b, m, g, 1, hts[g])
                        if prev is not None:
                            g2_chunk(prev[0], prev[1], prev[2], pot, g)
                else:
                    # halves-first order so own-GEMM2 can start early
                    for g in range(G):
                        g1_half(b, m, g, 0, hts[g])
                        if g < 2 and prev is not None:
                            g2_chunk(prev[0], prev[1], prev[2], pot, 2 * g)
                            g2_chunk(prev[0], prev[1], prev[2], pot, 2 * g + 1)
                    for g in range(G):
                        g1_half(b, m, g, 1, hts[g])
                # x prefetch for batch b+1 spread over early iterations
                if b + 1 < BPC and m in (1, 2):
                    for g in range(2):
                        load_x(b + 1, 2 * (m - 1) + g)
                prev = (b, m, hts)
            # tail: GEMM2 of the last iteration
            pot = opool.tile([128, T], F32, tag="o", name="pot")
            for c in range(T // CH):
                g2_chunk(prev[0], prev[1], prev[2], pot, c, fin=True)

    nc.compile()
    return nc


def get_nc(mm_dt=None):
    mm_dt = MM_DT if mm_dt is None else mm_dt
    if mm_dt not in _compiled:
        _compiled[mm_dt] = _build(mm_dt)
    return _compiled[mm_dt]


def prep_inputs(x, x_mask, w1, b1, w2, b2):
    """Host-side layout prep. Returns per-core in_maps."""
    import ml_dtypes

    bf16 = ml_dtypes.bfloat16
    x = np.asarray(x, dtype=np.float32)
    w1 = np.asarray(w1, dtype=np.float32)
    b1 = np.asarray(b1, dtype=np.float32)
    w2 = np.asarray(w2, dtype=np.float32)
    b2 = np.asarray(b2, dtype=np.float32)

    # w1 [H, CIN/G] -> lhsT blocks [i, (m, g, o)]
    w1r = w1.reshape(G, MB, 128, CIN // G)          # g, m, o, i
    w1t = np.transpose(w1r, (3, 1, 0, 2)).reshape(128, G * MB * 128)
    # w2 [COUT, H/G] -> lhsT blocks [r, (g2, g, o)]; GEMM2 group g2
    # contracts h tile (g, m=g2) row r against w2[g2*128+o, r*G+g]
    # (channel shuffle pre-applied).
    w2r = w2.reshape(G, 128, 128, G)                # g2, o, r, g
    w2t = np.transpose(w2r, (2, 0, 3, 1)).reshape(128, G * G * 128)
    wpk = np.ascontiguousarray(
        np.concatenate([w1t, w2t], axis=1).astype(bf16)
    )
    b1t = b1.reshape(G, MB, 128).transpose(2, 1, 0).reshape(128, G * MB)
    b2t = b2.reshape(G, 128).T
    bpk = np.ascontiguousarray(
        np.concatenate([b1t, b2t], axis=1).astype(np.float32)
    )

    xr = np.ascontiguousarray(
        x.reshape(N_CORES, BPC * G, 128, T).astype(bf16)
    )

    in_maps = []
    for k in range(N_CORES):
        in_maps.append({"xs": xr[k], "wpk": wpk, "bpk": bpk})
    return in_maps


def assemble_output(results):
    """results: list of 8 dicts with 'outs' [BPC*G, 128, T]."""
    parts = [r["outs"].reshape(BPC, G * 128, T) for r in results]
    return np.concatenate(parts, axis=0).astype(np.float32)


def kernel(x, x_mask, w1, b1, w2, b2, n_groups):
    assert int(n_groups) == G
    import os

    # NTFF tracing needs antenv.axon_hooks, absent on this image; make
    # sure an inherited BASS_TRACE can't push us onto that path.
    os.environ["BASS_NEVER_TRACE"] = "1"

    x = np.asarray(x, dtype=np.float32)
    x_mask = np.asarray(x_mask, dtype=np.float32)
    trivial_mask = bool(np.all(x_mask == 1.0))
    if not trivial_mask:
        # mask is per-(b,t): it commutes with the pointwise convs, so
        # exact host-side pre/post multiply preserves semantics.
        x = x * x_mask

    nc = get_nc()
    in_maps = prep_inputs(x, x_mask, w1, b1, w2, b2)
    res = bass_utils.run_bass_kernel_spmd(
        nc, in_maps, core_ids=list(range(N_CORES))
    )
    out = assemble_output(res.results)
    if not trivial_mask:
        out = out * x_mask
    return out


# revision 8
# speedup vs baseline: 1.0044x; 1.0044x over previous
"""Trainium2 Bass kernel: grouped-pointwise FFN with channel shuffle.

Computes (per batch b, all ops pointwise in T):
    h   = W1_grouped @ (x * mask) + b1          # G=4 block-diagonal GEMM
    h   = channel_shuffle(h, G)
    h   = gelu(h)                               # exact erf gelu
    out = (W2_grouped @ h + b2) * mask

Sharding: data-parallel over batch B=16 across 8 cores (2 batches/core).
Weights are replicated; no collectives.

The spec pins x_mask to all-ones, so the kernel drops the mask path on
device; if a caller ever passes a non-trivial mask it is applied exactly
on the host (x*mask pre, out*mask post) which commutes with the kernel.

Engine budget per core (pace analysis):
  ACT (gelu, 1.2GHz, dtype-independent): 64 ops x [128,1024] ~ 68.5us  <- pace
  PE  (256 matmuls x 512 free, bf16 1cyc/row @2.4GHz): ~54.6us
  DVE (GEMM2 drain +bias): 32 x [128,512] ~ 22.5us
  DMA (bf16 in 5.2MB, f32 out 8.4MB): ~41us wire
ACT is the critical engine; the schedule keeps it streaming back-to-back:
GEMM1 halves feed a 3-buf [128,1024] PSUM pool (6 banks), GEMM2 uses a
2-buf [128,512] pool (2 banks), GEMM2 of iteration i interleaves with
GEMM1 of iteration i+1 (lag 1) except the last iteration which
interleaves within itself to shorten the tail.

Channel shuffle is free: GEMM2's weight blocks are pre-gathered on the
host so GEMM2 group g2 contracts directly over GEMM1's (g, m=g2) tiles.

All matmul operands are bf16 (same PE rate as fp32r; half the DMA bytes
and SBUF); PSUM stays fp32, gelu output h is bf16, final out fp32.
"""

import numpy as np

import concourse.mybir as mybir
import concourse.tile as tile
from concourse import bacc
from concourse import bass_utils

F32 = mybir.dt.float32
BF16 = mybir.dt.bfloat16

N_CORES = 8
B, CIN, T = 16, 512, 2048
H, COUT, G = 2048, 512, 4
BPC = B // N_CORES        # batches per core
MB = (H // G) // 128      # 4 output-channel blocks per group in GEMM1
CH = 512                  # matmul free dim (1 PSUM bank)
AW = 1024                 # ACT op width (2 PSUM banks)

MM_DT = BF16

_compiled = {}


def _build(mm_dt):
    nc = bacc.Bacc(
        "TRN2", target_bir_lowering=False, debug=False, num_devices=N_CORES
    )
    xs = nc.dram_tensor("xs", [BPC * G, 128, T], mm_dt, kind="ExternalInput").ap()
    # wpk cols: w1t [(m, g, o)-major, 2048] then w2t [(g2, g, o)-major, 2048]
    wpk = nc.dram_tensor("wpk", [128, 2 * G * MB * 128], mm_dt, kind="ExternalInput").ap()
    # bpk cols: b1t [m*G+g, 16] then b2t [g2, 4]
    bpk = nc.dram_tensor("bpk", [128, G * MB + G], F32, kind="ExternalInput").ap()
    outs = nc.dram_tensor("outs", [BPC * G, 128, T], F32, kind="ExternalOutput").ap()

    with tile.TileContext(nc) as tc:
        with (
            tc.tile_pool(name="consts", bufs=1) as cpool,
            tc.tile_pool(name="xp", bufs=BPC * G) as xpool,
            tc.tile_pool(name="hp", bufs=2 * G) as hpool,
            tc.tile_pool(name="op", bufs=2) as opool,
            tc.tile_pool(name="ps1p", bufs=3, space="PSUM") as ps1pool,
            tc.tile_pool(name="ps2p", bufs=2, space="PSUM") as ps2pool,
        ):
            w_sb = cpool.tile([128, 2 * G * MB * 128], mm_dt)
            b_sb = cpool.tile([128, G * MB + G], F32)
            wup = cpool.tile([1, CH], mm_dt)
            dmy = cpool.tile([1, 8], mm_dt)
            x_sb = [[None] * G for _ in range(BPC)]

            # memset-fed warm-up source + dummy gelu: the dummy pulls the
            # Gelu ACT_TABLE_LOAD (1.3us) off ACT#1's critical path.
            nc.gpsimd.memset(wup.bitcast(mybir.dt.uint16), 0x3F80)
            nc.scalar.activation(
                dmy, wup[:, 0:8], mybir.ActivationFunctionType.Gelu, scale=1.0
            )

            # head DMAs, first-need order. sync HWDGE ring: w1 m=0 and
            # x(0,0) in chunks so GEMM1 unblocks progressively, then
            # x(0,1) ahead of the remaining weights. gpsimd ring takes
            # x(0,2)/x(0,3) in parallel.
            xt = [
                xpool.tile([128, T], mm_dt, tag="x", name="xt")
                for _ in range(G)
            ]
            nc.sync.dma_start(w_sb[:, 0 : G * 128], wpk[:, 0 : G * 128])
            nc.sync.dma_start(xt[0][:, 0:CH], xs[0][:, 0:CH])
            nc.sync.dma_start(xt[0][:, CH : 2 * CH], xs[0][:, CH : 2 * CH])
            nc.sync.dma_start(b_sb, bpk)
            nc.gpsimd.dma_start(xt[2][:, 0:AW], xs[2][:, 0:AW])
            nc.sync.dma_start(xt[0][:, 2 * CH : T], xs[0][:, 2 * CH : T])
            nc.gpsimd.dma_start(xt[2][:, AW:T], xs[2][:, AW:T])
            nc.sync.dma_start(xt[1][:, 0:AW], xs[1][:, 0:AW])
            nc.gpsimd.dma_start(xt[3][:, 0:AW], xs[3][:, 0:AW])
            nc.sync.dma_start(xt[1][:, AW:T], xs[1][:, AW:T])
            nc.gpsimd.dma_start(xt[3][:, AW:T], xs[3][:, AW:T])
            nc.sync.dma_start(
                w_sb[:, G * 128 : G * MB * 128], wpk[:, G * 128 : G * MB * 128]
            )
            nc.sync.dma_start(
                w_sb[:, G * MB * 128 :], wpk[:, G * MB * 128 :]
            )
            for g in range(G):
                x_sb[0][g] = xt[g]

            # PE p-state warm-up while the head DMAs stream: memset-fed
            # K=1 matmuls, no data dependencies.
            wps = ps2pool.tile([128, CH], F32, tag="ps2", name="wps")
            for _ in range(5):
                nc.tensor.matmul(wps, wup[:, 0:128], wup, start=True, stop=True)

            def load_x(b, g):
                xt = xpool.tile([128, T], mm_dt, tag="x", name="xt")
                nc.sync.dma_start(xt, xs[b * G + g])
                x_sb[b][g] = xt

            def g1_half(b, m, g, half, ht):
                # one [128,1024] PSUM tile: 2 matmuls + fused gelu/bias
                ps1 = ps1pool.tile([128, AW], F32, tag="ps1", name="ps1")
                w_ap = w_sb[:, (m * G + g) * 128 : (m * G + g + 1) * 128]
                for c2 in range(AW // CH):
                    c = half * (AW // CH) + c2
                    nc.tensor.matmul(
                        ps1[:, c2 * CH : (c2 + 1) * CH],
                        w_ap,
                        x_sb[b][g][:, c * CH : (c + 1) * CH],
                        start=True, stop=True,
                    )
                nc.scalar.activation(
                    ht[:, half * AW : (half + 1) * AW],
                    ps1,
                    mybir.ActivationFunctionType.Gelu,
                    bias=b_sb[:, m * G + g : m * G + g + 1],
                    scale=1.0,
                )

            def g2_chunk(b, g2, hts, ot, c, fin=False):
                cs = slice(c * CH, (c + 1) * CH)
                ps2 = ps2pool.tile([128, CH], F32, tag="ps2", name="ps2")
                for g in range(G):
                    wo = G * MB * 128 + (g2 * G + g) * 128
                    nc.tensor.matmul(
                        ps2,
                        w_sb[:, wo : wo + 128],
                        hts[g][:, cs],
                        start=(g == 0), stop=(g == G - 1),
                    )
                # out = psum + b2 (per-partition scalar) on DVE
                nc.vector.tensor_scalar_add(
                    ot[:, cs],
                    ps2,
                    b_sb[:, G * MB + g2 : G * MB + g2 + 1],
                )
                if fin:
                    # tail: per-chunk stores, alternating rings so the last
                    # two transfers run in parallel
                    ring = nc.sync if c % 2 == 0 else nc.gpsimd
                    ring.dma_start(outs[b * G + g2][:, cs], ot[:, cs])
                elif c % 2 == 1:  # steady state: store half-tiles
                    os_ = slice((c - 1) * CH, (c + 1) * CH)
                    nc.gpsimd.dma_start(outs[b * G + g2][:, os_], ot[:, os_])

            # pipeline: GEMM2 of iter i-1 interleaves with GEMM1 of iter i;
            # the last iteration interleaves its own GEMM2 (after the h
            # halves it needs) to shorten the tail.
            NIT = BPC * MB
            prev = None
            for it in range(NIT):
                b, m = divmod(it, MB)
                last = it == NIT - 1
                hts = [
                    hpool.tile([128, T], mm_dt, tag="h", name="ht")
                    for _ in range(G)
                ]
                if prev is not None:
                    pot = opool.tile([128, T], F32, tag="o", name="pot")
                if not last:
                    for g in range(G):
                        g1_half(b, m, g, 0, hts[g])
                        g1_half(b, m, g, 1, hts[g])
                        if prev is not None:
                            g2_chunk(prev[0], prev[1], prev[2], pot, g)
                else:
                    # halves-first order so own-GEMM2 can start early
                    for g in range(G):
                        g1_half(b, m, g, 0, hts[g])
                        if g < 2 and prev is not None:
                            g2_chunk(prev[0], prev[1], prev[2], pot, 2 * g)
                            g2_chunk(prev[0], prev[1], prev[2], pot, 2 * g + 1)
                    for g in range(G):
                        g1_half(b, m, g, 1, hts[g])
                # x prefetch for batch b+1 spread over early iterations
                if b + 1 < BPC and m in (1, 2):
                    for g in range(2):
                        load_x(b + 1, 2 * (m - 1) + g)
                prev = (b, m, hts)
            # tail: GEMM2 of the last iteration
            pot = opool.tile([128, T], F32, tag="o", name="pot")
            for c in range(T // CH):
                g2_chunk(prev[0], prev[1], prev[2], pot, c, fin=True)

    nc.compile()
    return nc


def get_nc(mm_dt=None):
    mm_dt = MM_DT if mm_dt is None else mm_dt
    if mm_dt not in _compiled:
        _compiled[mm_dt] = _build(mm_dt)
    return _compiled[mm_dt]


def prep_inputs(x, x_mask, w1, b1, w2, b2):
    """Host-side layout prep. Returns per-core in_maps."""
    import ml_dtypes

    bf16 = ml_dtypes.bfloat16
    x = np.asarray(x, dtype=np.float32)
    w1 = np.asarray(w1, dtype=np.float32)
    b1 = np.asarray(b1, dtype=np.float32)
    w2 = np.asarray(w2, dtype=np.float32)
    b2 = np.asarray(b2, dtype=np.float32)

    # w1 [H, CIN/G] -> lhsT blocks [i, (m, g, o)]
    w1r = w1.reshape(G, MB, 128, CIN // G)          # g, m, o, i
    w1t = np.transpose(w1r, (3, 1, 0, 2)).reshape(128, G * MB * 128)
    # w2 [COUT, H/G] -> lhsT blocks [r, (g2, g, o)]; GEMM2 group g2
    # contracts h tile (g, m=g2) row r against w2[g2*128+o, r*G+g]
    # (channel shuffle pre-applied).
    w2r = w2.reshape(G, 128, 128, G)                # g2, o, r, g
    w2t = np.transpose(w2r, (2, 0, 3, 1)).reshape(128, G * G * 128)
    wpk = np.ascontiguousarray(
        np.concatenate([w1t, w2t], axis=1).astype(bf16)
    )
    b1t = b1.reshape(G, MB, 128).transpose(2, 1, 0).reshape(128, G * MB)
    b2t = b2.reshape(G, 128).T
    bpk = np.ascontiguousarray(
        np.concatenate([b1t, b2t], axis=1).astype(np.float32)
    )

    xr = np.ascontiguousarray(
        x.reshape(N_CORES, BPC * G, 128, T).astype(bf16)
    )

    in_maps = []
    for k in range(N_CORES):
        in_maps.append({"xs": xr[k], "wpk": wpk, "bpk": bpk})
    return in_maps


def assemble_output(results):
    """results: list of 8 dicts with 'outs' [BPC*G, 128, T]."""
    parts = [r["outs"].reshape(BPC, G * 128, T) for r in results]
    return np.concatenate(parts, axis=0).astype(np.float32)


def kernel(x, x_mask, w1, b1, w2, b2, n_groups):
    assert int(n_groups) == G
    import os

    # NTFF tracing needs antenv.axon_hooks, absent on this image; make
    # sure an inherited BASS_TRACE can't push us onto that path.
    os.environ["BASS_NEVER_TRACE"] = "1"

    x = np.asarray(x, dtype=np.float32)
    x_mask = np.asarray(x_mask, dtype=np.float32)
    trivial_mask = bool(np.all(x_mask == 1.0))
    if not trivial_mask:
        # mask is per-(b,t): it commutes with the pointwise convs, so
        # exact host-side pre/post multiply preserves semantics.
        x = x * x_mask

    nc = get_nc()
    in_maps = prep_inputs(x, x_mask, w1, b1, w2, b2)
    res = bass_utils.run_bass_kernel_spmd(
        nc, in_maps, core_ids=list(range(N_CORES))
    )
    out = assemble_output(res.results)
    if not trivial_mask:
        out = out * x_mask
    return out
